# revision 36
# baseline (speedup 1.0000x reference)
"""Trainium2 Bass kernel for nn_Encoder_57913339020010 (sparse_attention).

Outputs (matching reference):
  scores  [4, 2048, 2048] f32 = tril(gate*scale*exp(-decay*|ti-tj|), -1)
  hidden  [4, 2048, 64]   f32 = concat(positional-embedding, type-embedding)
  t_diff  [4, 2048, 2048] f32 = |t_j - t_i|

Sharding: 8 cores = 4 batches x 2 row-halves. Each core owns 8 row-chunks of
128 rows, interleaved as pairs (k, 15-k) so both halves of a batch do equal
causal work. The program is SPMD-uniform: chunk widths are padded to fixed
per-slot widths; per-core differences flow in via input tensors only.

Core algebra: gate/scale/decay depend on (type_i, type_j) only (21 types), so
they collapse to 21x21 lookup tables computed once per core, then expanded to
[128, N] pair tiles with one-hot matmuls on the tensor engine (exact for 0/1
weights). Per-element work is just |t_j-t_i| (ACT abs), one exp, two muls.
"""

import os
import sys

sys.path.insert(0, "/opt/trn_rl_repo")

import numpy as np

B, L, D, T = 4, 2048, 32, 21
P = 128  # partition rows per chunk / slot
NSLOT = 8
NCORES = 8
CHUNKS = {0: [0, 2, 4, 6, 9, 11, 13, 15], 1: [1, 3, 5, 7, 8, 10, 12, 14]}
SLOTW = [256, 512, 768, 1024, 1280, 1536, 1792, 2048]
PI = float(np.pi)

_BUILT = {}


def _apply_walrus_multiwait_patch():
    """This container's walrus encodes at most ONE sync-wait per instruction.
    Tile attaches all cross-engine waits to one instruction, so split the
    extras onto engine-matched NoOps placed just before, and fan the tail
    drain's global-clock waits across chained drains."""
    if _BUILT.get("patched"):
        return
    _BUILT["patched"] = True
    import bass_rust
    import concourse.tile as tile
    from concourse.vector_clock import ScopedClock

    ctr = [0]

    def mk_wait_nop(engine, wait):
        ctr[0] += 1
        nop = bass_rust.InstNoOp(name=f"I-wsplit-{ctr[0]}", ins=[], outs=[])
        nop.engine = engine
        nop.sync_info = bass_rust.SyncInfo(on_wait=[wait], on_update=[])
        return nop

    orig_lower = tile.TileContext._lower_ordered_insts

    def split_lower(self, ordered):
        for bb_name, insts in list(ordered.items()):
            out = []
            for inst in insts:
                si = inst.sync_info
                if si is not None and si.on_wait and len(si.on_wait) > 1:
                    waits = list(si.on_wait)
                    for w in waits[:-1]:
                        out.append(mk_wait_nop(inst.engine, w))
                    inst.sync_info = bass_rust.SyncInfo(
                        on_wait=[waits[-1]], on_update=list(si.on_update or [])
                    )
                out.append(inst)
            ordered[bb_name] = out
        return orig_lower(self, ordered)

    def split_drain(self, tick_clock, wait_clock):
        nc = self.nc
        drain_inst = nc.sync.drain()
        wait_clock.add_sem_waits(
            drain_inst.ins, ScopedClock({None: tick_clock.global_clock})
        )
        si = drain_inst.ins.sync_info
        if si is not None and si.on_wait and len(si.on_wait) > 1:
            waits = list(si.on_wait)
            drain_inst.ins.sync_info = bass_rust.SyncInfo(
                on_wait=[waits[0]], on_update=list(si.on_update or [])
            )
            for w in waits[1:]:
                d2 = nc.sync.drain()
                d2.ins.sync_info = bass_rust.SyncInfo(on_wait=[w], on_update=[])
        nc.all_engine_barrier()
        assert self.sems is not None
        popped = nc._tile_sem_poison_stack.pop()
        assert popped is self._sem_poison
        nc.clear_and_free_semaphores(list(self.sems.allocated().values()))
        nc.all_engine_barrier()

    tile.TileContext._lower_ordered_insts = split_lower
    tile.TileContext._drain_and_barrier = split_drain


def _jtiles(w):
    out = []
    o = 0
    while o < w:
        n = min(512, w - o)
        out.append((o, n))
        o += n
    return out


def _build_nc(for_sim=False):
    import concourse.bass as bass
    import concourse.tile as tile
    from concourse import mybir

    _apply_walrus_multiwait_patch()

    F32 = mybir.dt.float32
    I32 = mybir.dt.int32
    AF = mybir.ActivationFunctionType
    OP = mybir.AluOpType

    nc = bass.Bass("TRN2", target_bir_lowering=not for_sim, debug=False)

    din = lambda n, s, d=F32: nc.dram_tensor(n, s, d, kind="ExternalInput").ap()
    dout = lambda n, s: nc.dram_tensor(n, s, F32, kind="ExternalOutput").ap()

    t_row = din("t_row", [1, L])
    t_icol = din("t_icol", [P, NSLOT])
    ty_row = din("ty_row", [1, L], I32)
    ty_isl = din("ty_isl", [1, NSLOT * P], I32)
    table = din("table", [T, D])
    ttT = din("ttT", [D, T])
    w6 = din("w6", [D, 6])
    bvec = din("bvec", [1, 3])
    wt2 = din("wt2", [1, D])
    arc = din("arc", [NSLOT * P, D])
    icolg = din("icolg", [P, NSLOT])
    iota = din("iota21", [T, 1])
    iota2 = din("iota42", [2 * T, 1])
    ident = din("ident21", [T, T])

    sc_sh = dout("sc_sh", [NSLOT * P, L])
    td_sh = dout("td_sh", [NSLOT * P, L])
    hid_sh = dout("hid_sh", [NSLOT * P, 2 * D])

    with tile.TileContext(nc) as tc:
        import contextlib

        with contextlib.ExitStack() as ctx:
            ones = ctx.enter_context(tc.tile_pool(name="singles", bufs=1))
            tdp = ctx.enter_context(tc.tile_pool(name="td", bufs=3))
            scp = ctx.enter_context(tc.tile_pool(name="sc", bufs=3))
            ewp = ctx.enter_context(tc.tile_pool(name="ew", bufs=6))
            smp = ctx.enter_context(tc.tile_pool(name="small", bufs=2))
            pgp = ctx.enter_context(tc.tile_pool(name="ptab", bufs=3))
            psb = ctx.enter_context(tc.tile_pool(name="psbig", bufs=2, space="PSUM"))
            psd = ctx.enter_context(tc.tile_pool(name="psdc", bufs=1, space="PSUM"))
            pss = ctx.enter_context(tc.tile_pool(name="pssmall", bufs=2, space="PSUM"))

            # ---------------- setup loads ----------------
            tj_b = ones.tile([P, L], F32)
            nc.sync.dma_start(out=tj_b, in_=t_row.to_broadcast((P, L)))
            ticol = ones.tile([P, NSLOT], F32)
            nc.sync.dma_start(out=ticol, in_=t_icol)
            nticol = ones.tile([P, NSLOT], F32)
            nc.vector.tensor_scalar(out=nticol, in0=ticol, scalar1=-1.0, scalar2=None, op0=OP.mult)

            BF16 = mybir.dt.bfloat16
            # one-hot over j, duplicated along K for the hi/lo split (K=42)
            tyb = ones.tile([2 * T, L], I32)
            nc.sync.dma_start(out=tyb, in_=ty_row.to_broadcast((2 * T, L)))
            tyf = ones.tile([2 * T, L], F32)
            nc.vector.tensor_copy(out=tyf, in_=tyb)
            iot = ones.tile([T, 1], F32)
            nc.sync.dma_start(out=iot, in_=iota)
            iot2 = ones.tile([2 * T, 1], F32)
            nc.sync.dma_start(out=iot2, in_=iota2)
            ohj2 = ones.tile([2 * T, L], BF16)
            nc.vector.tensor_scalar(out=ohj2, in0=tyf, scalar1=iot2, scalar2=None, op0=OP.is_equal)

            tyib = ones.tile([T, NSLOT * P], I32)
            nc.sync.dma_start(out=tyib, in_=ty_isl.to_broadcast((T, NSLOT * P)))
            tyif = ones.tile([T, NSLOT * P], F32)
            nc.vector.tensor_copy(out=tyif, in_=tyib)
            ohi = ones.tile([T, NSLOT * P], F32)
            nc.vector.tensor_scalar(out=ohi, in0=tyif, scalar1=iot, scalar2=None, op0=OP.is_equal)
            ohib = ones.tile([T, NSLOT * P], BF16)
            nc.vector.tensor_copy(out=ohib, in_=ohi)

            tbl = ones.tile([T, D], F32)
            nc.sync.dma_start(out=tbl, in_=table)
            ttT_t = ones.tile([D, T], F32)
            nc.sync.dma_start(out=ttT_t, in_=ttT)
            w6_t = ones.tile([D, 6], F32)
            nc.sync.dma_start(out=w6_t, in_=w6)
            bv = ones.tile([T, 3], F32)
            nc.sync.dma_start(out=bv, in_=bvec.to_broadcast((T, 3)))
            wt2b = ones.tile([P, D], F32)
            nc.sync.dma_start(out=wt2b, in_=wt2.to_broadcast((P, D)))
            arct = ones.tile([P, NSLOT, D], F32)
            nc.sync.dma_start(
                out=arct,
                in_=bass.AP(tensor=arc.tensor, offset=arc.offset,
                            ap=[[D, P], [P * D, NSLOT], [1, D]]),
            )
            zerot = ones.tile([P, L - 256], F32)
            nc.vector.memset(zerot, 0.0)

            # causal masks computed on-device: mask[p, q] = (Ws-256+q < i_glob)
            icg = ones.tile([P, NSLOT], F32)
            nc.sync.dma_start(out=icg, in_=icolg)
            q256i = ones.tile([P, 256], I32)
            nc.gpsimd.iota(q256i, pattern=[[1, 256]], base=0, channel_multiplier=0)
            q256f = ones.tile([P, 256], F32)
            nc.vector.tensor_copy(out=q256f, in_=q256i)
            maskt = ones.tile([P, NSLOT, 256], F32)
            for s in range(NSLOT):
                th = smp.tile([P, 1], F32, tag="maskth")
                nc.vector.tensor_scalar(out=th, in0=icg[:, s : s + 1],
                                        scalar1=float(SLOTW[s] - 256), scalar2=None,
                                        op0=OP.subtract)
                nc.vector.tensor_scalar(out=maskt[:, s, :], in0=q256f, scalar1=th,
                                        scalar2=None, op0=OP.is_lt)

            # ---------------- positional embedding (sin phase first) ----------------
            # z[p, s, k] = t_i[p, s] * Wt2[k] + arc[p, s, k]; pe = sign * sin(reduced z)
            zall = smp.tile([P, NSLOT, D], F32, tag="pe")
            for s in range(NSLOT):
                nc.vector.tensor_scalar(out=zall[:, s, :], in0=wt2b,
                                        scalar1=ticol[:, s : s + 1], scalar2=None, op0=OP.mult)
            nc.vector.tensor_tensor(out=zall, in0=zall, in1=arct, op=OP.add)
            yf = smp.tile([P, NSLOT * D], F32, tag="pe2")
            zfl = zall.rearrange("p s k -> p (s k)")
            nc.vector.tensor_scalar(out=yf, in0=zfl, scalar1=1.0 / PI, scalar2=None, op0=OP.mult)
            ni = smp.tile([P, NSLOT * D], I32, tag="pe3")
            nc.vector.tensor_copy(out=ni, in_=yf)  # f32->i32 rounds to nearest
            nf = smp.tile([P, NSLOT * D], F32, tag="pe4")
            nc.vector.tensor_copy(out=nf, in_=ni)
            nc.vector.tensor_scalar(out=nf, in0=nf, scalar1=-PI, scalar2=None, op0=OP.mult)
            rr = smp.tile([P, NSLOT * D], F32, tag="pe5")
            nc.vector.tensor_tensor(out=rr, in0=zfl, in1=nf, op=OP.add)
            nb = smp.tile([P, NSLOT * D], I32, tag="pe6")
            nc.vector.tensor_scalar(out=nb, in0=ni, scalar1=1, scalar2=None, op0=OP.bitwise_and)
            sgn = smp.tile([P, NSLOT * D], F32, tag="pe7")
            nc.vector.tensor_copy(out=sgn, in_=nb)
            nc.vector.tensor_scalar(out=sgn, in0=sgn, scalar1=-2.0, scalar2=1.0, op0=OP.mult, op1=OP.add)
            sr = smp.tile([P, NSLOT * D], F32, tag="pe8")
            nc.scalar.activation(out=sr, in_=rr, func=AF.Sin, bias=0.0, scale=1.0)
            peall = ones.tile([P, NSLOT, D], F32)
            pefl = peall.rearrange("p s k -> p (s k)")
            nc.vector.tensor_tensor(out=pefl, in0=sr, in1=sgn, op=OP.mult)

            # ---------------- 21x21 tables (exp/ln set) ----------------
            proj_ps = pss.tile([T, 6], F32, tag="aux")
            nc.tensor.matmul(proj_ps, ttT_t, w6_t, start=True, stop=True)
            proj = ones.tile([T, 6], F32)
            nc.scalar.copy(out=proj, in_=proj_ps)
            id21 = ones.tile([T, T], F32)
            nc.sync.dma_start(out=id21, in_=ident)
            ones21 = ones.tile([1, T], F32)
            nc.vector.memset(ones21, 1.0)

            mx_list = []
            for x in range(3):  # g, s, d
                # u_x column -> row (identity matmul, M=1), then broadcast to 21 rows
                urow_ps = pss.tile([1, T], F32, tag="aux")
                nc.tensor.matmul(urow_ps, proj[:, x : x + 1], id21, start=True, stop=True)
                urow = smp.tile([1, T], F32, tag=f"urow{x}")
                nc.scalar.copy(out=urow, in_=urow_ps)
                ub_ps = pss.tile([T, T], F32, tag="aux")
                nc.tensor.matmul(ub_ps, ones21, urow, start=True, stop=True)
                mx = smp.tile([T, T], F32, tag=f"m{x}")
                nc.vector.tensor_scalar(out=mx, in0=ub_ps, scalar1=proj[:, 3 + x : 4 + x],
                                        scalar2=bv[:, x : x + 1], op0=OP.add, op1=OP.add)
                mx_list.append(mx)
            # G = 1/(1+exp(-m_g))
            eg = smp.tile([T, T], F32, tag="eg")
            nc.scalar.activation(out=eg, in_=mx_list[0], func=AF.Exp, bias=0.0, scale=-1.0)
            nc.vector.tensor_scalar(out=eg, in0=eg, scalar1=1.0, scalar2=None, op0=OP.add)
            G = smp.tile([T, T], F32, tag="G")
            nc.vector.reciprocal(out=G, in_=eg)
            # S = ln(1+exp(m_s)); Dc = ln(1+exp(m_d))
            SD = []
            for x in (1, 2):
                ex = smp.tile([T, T], F32, tag=f"ex{x}")
                nc.scalar.activation(out=ex, in_=mx_list[x], func=AF.Exp, bias=0.0, scale=1.0)
                nc.vector.tensor_scalar(out=ex, in0=ex, scalar1=1.0, scalar2=None, op0=OP.add)
                lx = smp.tile([T, T], F32, tag=f"lx{x}")
                nc.scalar.activation(out=lx, in_=ex, func=AF.Ln, bias=0.0, scale=1.0)
                SD.append(lx)
            gs_tbl = ones.tile([T, T], F32)
            nc.vector.tensor_tensor(out=gs_tbl, in0=G, in1=SD[0], op=OP.mult)
            dc_tbl = SD[1]

            # bf16 hi/lo split of each value table, concatenated along the
            # free axis -> lhsT [21, 42] so one K=42 matmul applies hi+lo.
            def split_cat(src, name):
                cat = ones.tile([T, 2 * T], BF16, tag=f"cat_{name}")
                nc.vector.tensor_copy(out=cat[:, 0:T], in_=src)
                hif = smp.tile([T, T], F32, tag=f"hif{name}")
                nc.vector.tensor_copy(out=hif, in_=cat[:, 0:T])
                lo = smp.tile([T, T], F32, tag=f"lo{name}")
                nc.vector.tensor_tensor(out=lo, in0=src, in1=hif, op=OP.subtract)
                nc.vector.tensor_copy(out=cat[:, T : 2 * T], in_=lo)
                return cat

            gs_cat = split_cat(gs_tbl, "gs")
            dc_cat = split_cat(dc_tbl, "dc")

            # ---------------- hidden vector phase ----------------
            hidall = ones.tile([P, NSLOT, 2 * D], F32)
            for s in range(NSLOT):
                emb_ps = pss.tile([P, D], F32, tag="aux")
                nc.tensor.matmul(emb_ps, ohi[:, s * P : (s + 1) * P], tbl, start=True, stop=True)
                nc.vector.tensor_copy(out=hidall[:, s, 0:D], in_=peall[:, s, :])
                nc.scalar.copy(out=hidall[:, s, D : 2 * D], in_=emb_ps)
            nc.gpsimd.dma_start(
                out=bass.AP(tensor=hid_sh.tensor, offset=hid_sh.offset,
                            ap=[[2 * D, P], [P * 2 * D, NSLOT], [1, 2 * D]]),
                in_=hidall,
            )

            # ---------------- main per-slot loop ----------------
            for s in range(NSLOT):
                W = SLOTW[s]
                sl = slice(s * P, (s + 1) * P)

                # t_diff full row block: |t_j - t_i| via ACT abs with bias=-t_i
                td = tdp.tile([P, L], F32, tag="td")
                nc.scalar.activation(out=td, in_=tj_b, func=AF.Abs,
                                     bias=nticol[:, s : s + 1], scale=1.0)
                nc.gpsimd.dma_start(out=td_sh[sl, :], in_=td)

                # per-row hi/lo value tables for this slot (out [42, 128])
                pgs_ps = pss.tile([2 * T, P], F32, tag="aux")
                nc.tensor.matmul(pgs_ps, gs_cat, ohib[:, sl], start=True, stop=True)
                pgs = pgp.tile([2 * T, P], BF16, tag="pgs_sb")
                nc.scalar.copy(out=pgs, in_=pgs_ps)
                pdc_ps = pss.tile([2 * T, P], F32, tag="aux")
                nc.tensor.matmul(pdc_ps, dc_cat, ohib[:, sl], start=True, stop=True)
                pdc = pgp.tile([2 * T, P], BF16, tag="pdc_sb")
                nc.scalar.copy(out=pdc, in_=pdc_ps)

                sc = scp.tile([P, L], F32, tag="sc")
                # pair two 512-wide matmuls into one 2-bank PSUM tile so the
                # DVE/ACT stages run once per 1024 cols (half the fixed costs
                # and cross-engine sem hops)
                for o in range(0, W, 1024):
                    n = min(1024, W - o)
                    gs_ps = psb.tile([P, 1024], F32, tag="gs")
                    dc_ps = psd.tile([P, 1024], F32, tag="dc")
                    for oo in range(0, n, 512):
                        nn_ = min(512, n - oo)
                        nc.tensor.matmul(gs_ps[:, oo : oo + nn_], pgs,
                                         ohj2[:, o + oo : o + oo + nn_], start=True, stop=True)
                        nc.tensor.matmul(dc_ps[:, oo : oo + nn_], pdc,
                                         ohj2[:, o + oo : o + oo + nn_], start=True, stop=True)
                    # m = dc * td in place in PSUM; ACT exp reads PSUM (faster src)
                    nc.vector.tensor_tensor(out=dc_ps[:, :n], in0=dc_ps[:, :n],
                                            in1=td[:, o : o + n], op=OP.mult)
                    e = ewp.tile([P, 1024], F32, tag="e")
                    nc.scalar.activation(out=e[:, :n], in_=dc_ps[:, :n], func=AF.Exp,
                                         bias=0.0, scale=-1.0)
                    nc.vector.tensor_tensor(out=sc[:, o : o + n], in0=gs_ps[:, :n],
                                            in1=e[:, :n], op=OP.mult)
                # causal mask on the last 256 computed columns (gpsimd: DVE is busy)
                nc.gpsimd.tensor_tensor(out=sc[:, W - 256 : W], in0=sc[:, W - 256 : W],
                                        in1=maskt[:, s, :], op=OP.mult)
                nc.sync.dma_start(out=sc_sh[sl, 0:W], in_=sc[:, 0:W])
                if W < L:
                    nc.sync.dma_start(out=sc_sh[sl, W:L], in_=zerot[:, 0 : L - W])

    return nc


def _host_inputs(inputs):
    """Build the 8 per-core input maps from full inputs (sharding + constant prep)."""
    event_type = np.asarray(inputs["event_type"])
    event_time = np.asarray(inputs["event_time"], dtype=np.float32)
    Wt = np.asarray(inputs["Wt"], dtype=np.float32)
    type_table = np.asarray(inputs["type_table"], dtype=np.float32)
    w_g = np.asarray(inputs["w_g"], dtype=np.float32)
    w_s = np.asarray(inputs["w_s"], dtype=np.float32)
    w_d = np.asarray(inputs["w_d"], dtype=np.float32)
    b_g = np.float32(inputs["b_g"])
    b_s = np.float32(inputs["b_s"])
    b_d = np.float32(inputs["b_d"])

    # constants
    div = np.exp(np.arange(0, D, 2, dtype=np.float64) * (-np.log(10000.0) / D))
    i_idx = np.arange(L, dtype=np.float64)[:, None]
    arc_sin = np.mod(i_idx * div, 2 * np.pi).astype(np.float32)  # [L, 16]
    arc_cos = np.mod(i_idx * div + np.pi / 2, 2 * np.pi).astype(np.float32)
    arc_full = np.concatenate([arc_sin, arc_cos], axis=1)  # [L, 32]
    iota21 = np.arange(T, dtype=np.float32).reshape(T, 1)
    w6 = np.stack([w_g[:D], w_s[:D], w_d[:D], w_g[D:], w_s[D:], w_d[D:]], axis=1)
    bvec = np.array([[b_g, b_s, b_d]], dtype=np.float32)
    wt2 = np.concatenate([Wt, Wt]).reshape(1, D).astype(np.float32)

    in_maps = []
    for c in range(NCORES):
        b, sub = c // 2, c % 2
        ch = CHUNKS[sub]
        rows = np.concatenate([np.arange(g * P, (g + 1) * P) for g in ch])
        t_icol = event_time[b][rows].reshape(NSLOT, P).T.copy()  # [128, 8]
        ty_isl = event_type[b][rows].reshape(1, NSLOT * P).astype(np.int32)
        arc_sl = arc_full[rows]  # [1024, 32]
        icolg = rows.reshape(NSLOT, P).T.astype(np.float32).copy()  # [128, 8]
        in_maps.append({
            "t_row": event_time[b].reshape(1, L),
            "t_icol": np.ascontiguousarray(t_icol),
            "ty_row": event_type[b].reshape(1, L).astype(np.int32),
            "ty_isl": ty_isl,
            "table": type_table,
            "ttT": np.ascontiguousarray(type_table.T),
            "w6": w6,
            "bvec": bvec,
            "wt2": wt2,
            "arc": arc_sl,
            "icolg": icolg,
            "iota21": iota21,
            "iota42": np.concatenate([iota21, iota21]).reshape(2 * T, 1),
            "ident21": np.eye(T, dtype=np.float32),
        })
    return in_maps


def _assemble(results):
    scores = np.zeros((B, L, L), dtype=np.float32)
    t_diff = np.zeros((B, L, L), dtype=np.float32)
    hidden = np.zeros((B, L, 2 * D), dtype=np.float32)
    for c in range(NCORES):
        b, sub = c // 2, c % 2
        r = results[c]
        for s, g in enumerate(CHUNKS[sub]):
            gsl = slice(g * P, (g + 1) * P)
            ssl = slice(s * P, (s + 1) * P)
            scores[b, gsl, :] = r["sc_sh"][ssl]
            t_diff[b, gsl, :] = r["td_sh"][ssl]
            hidden[b, gsl, :] = r["hid_sh"][ssl]
    return scores, hidden, t_diff


def _install_axon_trace_shims():
    """Test-only: provide the missing `antenv.axon_hooks` NTFF hook via the
    axon .so C ABI, and stub the S3 artifact upload."""
    if _BUILT.get("trace_shim"):
        return
    _BUILT["trace_shim"] = True
    import contextlib
    import ctypes
    import types

    try:
        from antenv.axon_hooks import get_axon_ntff_profile_hook  # noqa: F401

        return
    except ImportError:
        pass

    so_path = "/opt/axon/libaxon_pjrt.so"
    lib = ctypes.CDLL(so_path)
    if not hasattr(lib, "axon_start_nrt_profile"):
        return
    lib.axon_start_nrt_profile.argtypes = [
        ctypes.POINTER(ctypes.c_int64),
        ctypes.c_size_t,
    ]
    lib.axon_start_nrt_profile.restype = ctypes.c_int64
    lib.axon_stop_nrt_profile.argtypes = [ctypes.c_char_p]
    lib.axon_stop_nrt_profile.restype = ctypes.c_int64

    @contextlib.contextmanager
    def _hook(output_dir, device_ids):
        import jax

        jax.devices()
        if device_ids:
            ids = (ctypes.c_int64 * len(device_ids))(*device_ids)
            rc = lib.axon_start_nrt_profile(ids, len(device_ids))
        else:
            rc = lib.axon_start_nrt_profile(None, 0)
        if rc != 0:
            raise RuntimeError(f"axon_start_nrt_profile rc={rc}")
        try:
            yield
        finally:
            n = lib.axon_stop_nrt_profile(str(output_dir).encode())
            print(f"profile: {n} file(s) written to {output_dir}")

    import antenv

    mod = types.ModuleType("antenv.axon_hooks")
    mod.get_axon_ntff_profile_hook = lambda: _hook
    mod.set_axon_ntff_profile_hook = lambda h: None
    sys.modules["antenv.axon_hooks"] = mod
    antenv.axon_hooks = mod

    from concourse import bass_utils

    bass_utils.upload_artifacts = lambda tmpdir: "local://" + str(tmpdir)

    # hlo_convert binary is a broken symlink in this image; HLO annotation is
    # cosmetic, so degrade to no annotation instead of failing the trace.
    try:
        import gauge.trn_perfetto as _tp

        _orig_ght = _tp.TrnPerfettoConv.get_hlo_text

        def _safe_ght(self):
            try:
                return _orig_ght(self)
            except Exception:
                self.annotate_hlo = False
                return ""

        _tp.TrnPerfettoConv.get_hlo_text = _safe_ght
    except Exception:
        pass


def _run(inputs, trace=False, **trace_kwargs):
    from concourse.bass_utils import run_bass_kernel_spmd

    if trace:
        _install_axon_trace_shims()
    if "nc" not in _BUILT:
        _BUILT["nc"] = _build_nc(for_sim=False)
    nc = _BUILT["nc"]
    in_maps = _host_inputs(inputs)
    res = run_bass_kernel_spmd(nc, in_maps, list(range(NCORES)), trace=trace, **trace_kwargs)
    return _assemble(res.results), res


def kernel(**inputs):
    (scores, hidden, t_diff), _ = _run(inputs)
    return scores, hidden, t_diff


# revision 37
# speedup vs baseline: 1.1064x; 1.1064x over previous
"""Trainium2 Bass kernel for nn_Encoder_57913339020010 (sparse_attention).

Outputs (matching reference):
  scores  [4, 2048, 2048] f32 = tril(gate*scale*exp(-decay*|ti-tj|), -1)
  hidden  [4, 2048, 64]   f32 = concat(positional-embedding, type-embedding)
  t_diff  [4, 2048, 2048] f32 = |t_j - t_i|

Sharding: 8 cores = 4 batches x 2 row-halves. Each core owns 8 row-chunks of
128 rows, interleaved as pairs (k, 15-k) so both halves of a batch do equal
causal work. The program is SPMD-uniform: chunk widths are padded to fixed
per-slot widths; per-core differences flow in via input tensors only.

Core algebra: gate/scale/decay depend on (type_i, type_j) only (21 types), so
they collapse to 21x21 lookup tables computed once per core, then expanded to
[128, N] pair tiles with one-hot matmuls on the tensor engine (exact for 0/1
weights). Per-element work is just |t_j-t_i| (ACT abs), one exp, two muls.
"""

import os
import sys

sys.path.insert(0, "/opt/trn_rl_repo")

import numpy as np

B, L, D, T = 4, 2048, 32, 21
P = 128  # partition rows per chunk / slot
NSLOT = 8
NCORES = 8
CHUNKS = {0: [0, 2, 4, 6, 9, 11, 13, 15], 1: [1, 3, 5, 7, 8, 10, 12, 14]}
SLOTW = [256, 512, 768, 1024, 1280, 1536, 1792, 2048]
PI = float(np.pi)

_BUILT = {}


def _apply_walrus_multiwait_patch():
    """This container's walrus encodes at most ONE sync-wait per instruction.
    Tile attaches all cross-engine waits to one instruction, so split the
    extras onto engine-matched NoOps placed just before, and fan the tail
    drain's global-clock waits across chained drains."""
    if _BUILT.get("patched"):
        return
    _BUILT["patched"] = True
    import bass_rust
    import concourse.tile as tile
    from concourse.vector_clock import ScopedClock

    ctr = [0]

    def mk_wait_nop(engine, wait):
        ctr[0] += 1
        nop = bass_rust.InstNoOp(name=f"I-wsplit-{ctr[0]}", ins=[], outs=[])
        nop.engine = engine
        nop.sync_info = bass_rust.SyncInfo(on_wait=[wait], on_update=[])
        return nop

    orig_lower = tile.TileContext._lower_ordered_insts

    def split_lower(self, ordered):
        for bb_name, insts in list(ordered.items()):
            out = []
            for inst in insts:
                si = inst.sync_info
                if si is not None and si.on_wait and len(si.on_wait) > 1:
                    waits = list(si.on_wait)
                    for w in waits[:-1]:
                        out.append(mk_wait_nop(inst.engine, w))
                    inst.sync_info = bass_rust.SyncInfo(
                        on_wait=[waits[-1]], on_update=list(si.on_update or [])
                    )
                out.append(inst)
            ordered[bb_name] = out
        return orig_lower(self, ordered)

    def split_drain(self, tick_clock, wait_clock):
        nc = self.nc
        drain_inst = nc.sync.drain()
        wait_clock.add_sem_waits(
            drain_inst.ins, ScopedClock({None: tick_clock.global_clock})
        )
        si = drain_inst.ins.sync_info
        if si is not None and si.on_wait and len(si.on_wait) > 1:
            waits = list(si.on_wait)
            drain_inst.ins.sync_info = bass_rust.SyncInfo(
                on_wait=[waits[0]], on_update=list(si.on_update or [])
            )
            for w in waits[1:]:
                d2 = nc.sync.drain()
                d2.ins.sync_info = bass_rust.SyncInfo(on_wait=[w], on_update=[])
        nc.all_engine_barrier()
        assert self.sems is not None
        popped = nc._tile_sem_poison_stack.pop()
        assert popped is self._sem_poison
        nc.clear_and_free_semaphores(list(self.sems.allocated().values()))
        nc.all_engine_barrier()

    tile.TileContext._lower_ordered_insts = split_lower
    tile.TileContext._drain_and_barrier = split_drain


def _jtiles(w):
    out = []
    o = 0
    while o < w:
        n = min(512, w - o)
        out.append((o, n))
        o += n
    return out


def _build_nc(for_sim=False):
    import concourse.bass as bass
    import concourse.tile as tile
    from concourse import mybir

    _apply_walrus_multiwait_patch()

    F32 = mybir.dt.float32
    I32 = mybir.dt.int32
    AF = mybir.ActivationFunctionType
    OP = mybir.AluOpType

    nc = bass.Bass("TRN2", target_bir_lowering=not for_sim, debug=False)

    din = lambda n, s, d=F32: nc.dram_tensor(n, s, d, kind="ExternalInput").ap()
    dout = lambda n, s: nc.dram_tensor(n, s, F32, kind="ExternalOutput").ap()

    t_row = din("t_row", [1, L])
    t_icol = din("t_icol", [P, NSLOT])
    ty_row = din("ty_row", [1, L], I32)
    ty_isl = din("ty_isl", [1, NSLOT * P], I32)
    table = din("table", [T, D])
    ttT = din("ttT", [D, T])
    w6 = din("w6", [D, 6])
    bvec = din("bvec", [1, 3])
    wt2 = din("wt2", [1, D])
    arc = din("arc", [NSLOT * P, D])
    icolg = din("icolg", [P, NSLOT])
    iota = din("iota21", [T, 1])
    iota2 = din("iota42", [2 * T, 1])
    ident = din("ident21", [T, T])

    sc_sh = dout("sc_sh", [NSLOT * P, L])
    td_sh = dout("td_sh", [NSLOT * P, L])
    hid_sh = dout("hid_sh", [NSLOT * P, 2 * D])

    with tile.TileContext(nc) as tc:
        import contextlib

        with contextlib.ExitStack() as ctx:
            ones = ctx.enter_context(tc.tile_pool(name="singles", bufs=1))
            tdp = ctx.enter_context(tc.tile_pool(name="td", bufs=3))
            scp = ctx.enter_context(tc.tile_pool(name="sc", bufs=3))
            ewp = ctx.enter_context(tc.tile_pool(name="ew", bufs=6))
            smp = ctx.enter_context(tc.tile_pool(name="small", bufs=2))
            pgp = ctx.enter_context(tc.tile_pool(name="ptab", bufs=3))
            psb = ctx.enter_context(tc.tile_pool(name="psbig", bufs=3, space="PSUM"))
            pss = ctx.enter_context(tc.tile_pool(name="pssmall", bufs=2, space="PSUM"))

            # ---------------- setup loads ----------------
            tj_b = ones.tile([P, L], F32)
            nc.sync.dma_start(out=tj_b, in_=t_row.to_broadcast((P, L)))
            ticol = ones.tile([P, NSLOT], F32)
            nc.sync.dma_start(out=ticol, in_=t_icol)
            nticol = ones.tile([P, NSLOT], F32)
            nc.vector.tensor_scalar(out=nticol, in0=ticol, scalar1=-1.0, scalar2=None, op0=OP.mult)

            BF16 = mybir.dt.bfloat16
            # one-hot over j, duplicated along K for the hi/lo split (K=42)
            tyb = ones.tile([2 * T, L], I32)
            nc.sync.dma_start(out=tyb, in_=ty_row.to_broadcast((2 * T, L)))
            tyf = ones.tile([2 * T, L], F32)
            nc.vector.tensor_copy(out=tyf, in_=tyb)
            iot = ones.tile([T, 1], F32)
            nc.sync.dma_start(out=iot, in_=iota)
            iot2 = ones.tile([2 * T, 1], F32)
            nc.sync.dma_start(out=iot2, in_=iota2)
            ohj2 = ones.tile([2 * T, L], BF16)
            nc.vector.tensor_scalar(out=ohj2, in0=tyf, scalar1=iot2, scalar2=None, op0=OP.is_equal)

            tyib = ones.tile([T, NSLOT * P], I32)
            nc.sync.dma_start(out=tyib, in_=ty_isl.to_broadcast((T, NSLOT * P)))
            tyif = ones.tile([T, NSLOT * P], F32)
            nc.vector.tensor_copy(out=tyif, in_=tyib)
            ohi = ones.tile([T, NSLOT * P], F32)
            nc.vector.tensor_scalar(out=ohi, in0=tyif, scalar1=iot, scalar2=None, op0=OP.is_equal)
            ohib = ones.tile([T, NSLOT * P], BF16)
            nc.vector.tensor_copy(out=ohib, in_=ohi)

            tbl = ones.tile([T, D], F32)
            nc.sync.dma_start(out=tbl, in_=table)
            ttT_t = ones.tile([D, T], F32)
            nc.sync.dma_start(out=ttT_t, in_=ttT)
            w6_t = ones.tile([D, 6], F32)
            nc.sync.dma_start(out=w6_t, in_=w6)
            bv = ones.tile([T, 3], F32)
            nc.sync.dma_start(out=bv, in_=bvec.to_broadcast((T, 3)))
            wt2b = ones.tile([P, D], F32)
            nc.sync.dma_start(out=wt2b, in_=wt2.to_broadcast((P, D)))
            arct = ones.tile([P, NSLOT, D], F32)
            nc.sync.dma_start(
                out=arct,
                in_=bass.AP(tensor=arc.tensor, offset=arc.offset,
                            ap=[[D, P], [P * D, NSLOT], [1, D]]),
            )
            zerot = ones.tile([P, L - 256], F32)
            nc.vector.memset(zerot, 0.0)

            # causal masks computed on-device: mask[p, q] = (Ws-256+q < i_glob)
            icg = ones.tile([P, NSLOT], F32)
            nc.sync.dma_start(out=icg, in_=icolg)
            q256i = ones.tile([P, 256], I32)
            nc.gpsimd.iota(q256i, pattern=[[1, 256]], base=0, channel_multiplier=0)
            q256f = ones.tile([P, 256], F32)
            nc.vector.tensor_copy(out=q256f, in_=q256i)
            maskt = ones.tile([P, NSLOT, 256], F32)
            for s in range(NSLOT):
                th = smp.tile([P, 1], F32, tag="maskth")
                nc.vector.tensor_scalar(out=th, in0=icg[:, s : s + 1],
                                        scalar1=float(SLOTW[s] - 256), scalar2=None,
                                        op0=OP.subtract)
                nc.vector.tensor_scalar(out=maskt[:, s, :], in0=q256f, scalar1=th,
                                        scalar2=None, op0=OP.is_lt)

            # ---------------- positional embedding (sin phase first) ----------------
            # z[p, s, k] = t_i[p, s] * Wt2[k] + arc[p, s, k]; pe = sign * sin(reduced z)
            zall = smp.tile([P, NSLOT, D], F32, tag="pe")
            for s in range(NSLOT):
                nc.vector.tensor_scalar(out=zall[:, s, :], in0=wt2b,
                                        scalar1=ticol[:, s : s + 1], scalar2=None, op0=OP.mult)
            nc.vector.tensor_tensor(out=zall, in0=zall, in1=arct, op=OP.add)
            yf = smp.tile([P, NSLOT * D], F32, tag="pe2")
            zfl = zall.rearrange("p s k -> p (s k)")
            nc.vector.tensor_scalar(out=yf, in0=zfl, scalar1=1.0 / PI, scalar2=None, op0=OP.mult)
            ni = smp.tile([P, NSLOT * D], I32, tag="pe3")
            nc.vector.tensor_copy(out=ni, in_=yf)  # f32->i32 rounds to nearest
            nf = smp.tile([P, NSLOT * D], F32, tag="pe4")
            nc.vector.tensor_copy(out=nf, in_=ni)
            nc.vector.tensor_scalar(out=nf, in0=nf, scalar1=-PI, scalar2=None, op0=OP.mult)
            rr = smp.tile([P, NSLOT * D], F32, tag="pe5")
            nc.vector.tensor_tensor(out=rr, in0=zfl, in1=nf, op=OP.add)
            nb = smp.tile([P, NSLOT * D], I32, tag="pe6")
            nc.vector.tensor_scalar(out=nb, in0=ni, scalar1=1, scalar2=None, op0=OP.bitwise_and)
            sgn = smp.tile([P, NSLOT * D], F32, tag="pe7")
            nc.vector.tensor_copy(out=sgn, in_=nb)
            nc.vector.tensor_scalar(out=sgn, in0=sgn, scalar1=-2.0, scalar2=1.0, op0=OP.mult, op1=OP.add)
            sr = smp.tile([P, NSLOT * D], F32, tag="pe8")
            nc.scalar.activation(out=sr, in_=rr, func=AF.Sin, bias=0.0, scale=1.0)
            peall = ones.tile([P, NSLOT, D], F32)
            pefl = peall.rearrange("p s k -> p (s k)")
            nc.vector.tensor_tensor(out=pefl, in0=sr, in1=sgn, op=OP.mult)

            # ---------------- 21x21 tables (exp/ln set) ----------------
            proj_ps = pss.tile([T, 6], F32, tag="aux")
            nc.tensor.matmul(proj_ps, ttT_t, w6_t, start=True, stop=True)
            proj = ones.tile([T, 6], F32)
            nc.scalar.copy(out=proj, in_=proj_ps)
            id21 = ones.tile([T, T], F32)
            nc.sync.dma_start(out=id21, in_=ident)
            ones21 = ones.tile([1, T], F32)
            nc.vector.memset(ones21, 1.0)

            mx_list = []
            for x in range(3):  # g, s, d
                # u_x column -> row (identity matmul, M=1), then broadcast to 21 rows
                urow_ps = pss.tile([1, T], F32, tag="aux")
                nc.tensor.matmul(urow_ps, proj[:, x : x + 1], id21, start=True, stop=True)
                urow = smp.tile([1, T], F32, tag=f"urow{x}")
                nc.scalar.copy(out=urow, in_=urow_ps)
                ub_ps = pss.tile([T, T], F32, tag="aux")
                nc.tensor.matmul(ub_ps, ones21, urow, start=True, stop=True)
                mx = smp.tile([T, T], F32, tag=f"m{x}")
                nc.vector.tensor_scalar(out=mx, in0=ub_ps, scalar1=proj[:, 3 + x : 4 + x],
                                        scalar2=bv[:, x : x + 1], op0=OP.add, op1=OP.add)
                mx_list.append(mx)
            # G = 1/(1+exp(-m_g))
            eg = smp.tile([T, T], F32, tag="eg")
            nc.scalar.activation(out=eg, in_=mx_list[0], func=AF.Exp, bias=0.0, scale=-1.0)
            nc.vector.tensor_scalar(out=eg, in0=eg, scalar1=1.0, scalar2=None, op0=OP.add)
            G = smp.tile([T, T], F32, tag="G")
            nc.vector.reciprocal(out=G, in_=eg)
            # S = ln(1+exp(m_s)); Dc = ln(1+exp(m_d))
            SD = []
            for x in (1, 2):
                ex = smp.tile([T, T], F32, tag=f"ex{x}")
                nc.scalar.activation(out=ex, in_=mx_list[x], func=AF.Exp, bias=0.0, scale=1.0)
                nc.vector.tensor_scalar(out=ex, in0=ex, scalar1=1.0, scalar2=None, op0=OP.add)
                lx = smp.tile([T, T], F32, tag=f"lx{x}")
                nc.scalar.activation(out=lx, in_=ex, func=AF.Ln, bias=0.0, scale=1.0)
                SD.append(lx)
            gs_tbl = ones.tile([T, T], F32)
            nc.vector.tensor_tensor(out=gs_tbl, in0=G, in1=SD[0], op=OP.mult)
            dc_tbl = SD[1]

            # bf16 hi/lo split of each value table, concatenated along the
            # free axis -> lhsT [21, 42] so one K=42 matmul applies hi+lo.
            def split_cat(src, name):
                cat = ones.tile([T, 2 * T], BF16, tag=f"cat_{name}")
                nc.vector.tensor_copy(out=cat[:, 0:T], in_=src)
                hif = smp.tile([T, T], F32, tag=f"hif{name}")
                nc.vector.tensor_copy(out=hif, in_=cat[:, 0:T])
                lo = smp.tile([T, T], F32, tag=f"lo{name}")
                nc.vector.tensor_tensor(out=lo, in0=src, in1=hif, op=OP.subtract)
                nc.vector.tensor_copy(out=cat[:, T : 2 * T], in_=lo)
                return cat

            gs_cat = split_cat(gs_tbl, "gs")
            dc_cat = split_cat(dc_tbl, "dc")

            # ---------------- hidden vector phase ----------------
            hidall = ones.tile([P, NSLOT, 2 * D], F32)
            for s in range(NSLOT):
                emb_ps = pss.tile([P, D], F32, tag="aux")
                nc.tensor.matmul(emb_ps, ohi[:, s * P : (s + 1) * P], tbl, start=True, stop=True)
                nc.vector.tensor_copy(out=hidall[:, s, 0:D], in_=peall[:, s, :])
                nc.scalar.copy(out=hidall[:, s, D : 2 * D], in_=emb_ps)
            nc.gpsimd.dma_start(
                out=bass.AP(tensor=hid_sh.tensor, offset=hid_sh.offset,
                            ap=[[2 * D, P], [P * 2 * D, NSLOT], [1, 2 * D]]),
                in_=hidall,
            )

            # ---------------- main per-slot loop ----------------
            for s in range(NSLOT):
                W = SLOTW[s]
                sl = slice(s * P, (s + 1) * P)

                # t_diff full row block: |t_j - t_i| via ACT abs with bias=-t_i
                td = tdp.tile([P, L], F32, tag="td")
                nc.scalar.activation(out=td, in_=tj_b, func=AF.Abs,
                                     bias=nticol[:, s : s + 1], scale=1.0)
                nc.gpsimd.dma_start(out=td_sh[sl, :], in_=td)

                # per-row hi/lo value tables for this slot (out [42, 128])
                pgs_ps = pss.tile([2 * T, P], F32, tag="aux")
                nc.tensor.matmul(pgs_ps, gs_cat, ohib[:, sl], start=True, stop=True)
                pgs = pgp.tile([2 * T, P], BF16, tag="pgs_sb")
                nc.scalar.copy(out=pgs, in_=pgs_ps)
                pdc_ps = pss.tile([2 * T, P], F32, tag="aux")
                nc.tensor.matmul(pdc_ps, dc_cat, ohib[:, sl], start=True, stop=True)
                pdc = pgp.tile([2 * T, P], BF16, tag="pdc_sb")
                nc.scalar.copy(out=pdc, in_=pdc_ps)

                sc = scp.tile([P, L], F32, tag="sc")
                for (o, n) in _jtiles(W):
                    gs_ps = psb.tile([P, 512], F32, tag="gs")
                    nc.tensor.matmul(gs_ps[:, :n], pgs, ohj2[:, o : o + n], start=True, stop=True)
                    dc_ps = psb.tile([P, 512], F32, tag="dc")
                    nc.tensor.matmul(dc_ps[:, :n], pdc, ohj2[:, o : o + n], start=True, stop=True)
                    # m = dc * td in place in PSUM; ACT exp reads PSUM (faster src)
                    nc.vector.tensor_tensor(out=dc_ps[:, :n], in0=dc_ps[:, :n],
                                            in1=td[:, o : o + n], op=OP.mult)
                    e = ewp.tile([P, 512], F32, tag="e")
                    nc.scalar.activation(out=e[:, :n], in_=dc_ps[:, :n], func=AF.Exp,
                                         bias=0.0, scale=-1.0)
                    nc.vector.tensor_tensor(out=sc[:, o : o + n], in0=gs_ps[:, :n],
                                            in1=e[:, :n], op=OP.mult)
                # causal mask on the last 256 computed columns (gpsimd: DVE is busy)
                nc.gpsimd.tensor_tensor(out=sc[:, W - 256 : W], in0=sc[:, W - 256 : W],
                                        in1=maskt[:, s, :], op=OP.mult)
                nc.sync.dma_start(out=sc_sh[sl, 0:W], in_=sc[:, 0:W])
                if W < L:
                    nc.sync.dma_start(out=sc_sh[sl, W:L], in_=zerot[:, 0 : L - W])

    return nc


def _host_inputs(inputs):
    """Build the 8 per-core input maps from full inputs (sharding + constant prep)."""
    event_type = np.asarray(inputs["event_type"])
    event_time = np.asarray(inputs["event_time"], dtype=np.float32)
    Wt = np.asarray(inputs["Wt"], dtype=np.float32)
    type_table = np.asarray(inputs["type_table"], dtype=np.float32)
    w_g = np.asarray(inputs["w_g"], dtype=np.float32)
    w_s = np.asarray(inputs["w_s"], dtype=np.float32)
    w_d = np.asarray(inputs["w_d"], dtype=np.float32)
    b_g = np.float32(inputs["b_g"])
    b_s = np.float32(inputs["b_s"])
    b_d = np.float32(inputs["b_d"])

    # constants
    div = np.exp(np.arange(0, D, 2, dtype=np.float64) * (-np.log(10000.0) / D))
    i_idx = np.arange(L, dtype=np.float64)[:, None]
    arc_sin = np.mod(i_idx * div, 2 * np.pi).astype(np.float32)  # [L, 16]
    arc_cos = np.mod(i_idx * div + np.pi / 2, 2 * np.pi).astype(np.float32)
    arc_full = np.concatenate([arc_sin, arc_cos], axis=1)  # [L, 32]
    iota21 = np.arange(T, dtype=np.float32).reshape(T, 1)
    w6 = np.stack([w_g[:D], w_s[:D], w_d[:D], w_g[D:], w_s[D:], w_d[D:]], axis=1)
    bvec = np.array([[b_g, b_s, b_d]], dtype=np.float32)
    wt2 = np.concatenate([Wt, Wt]).reshape(1, D).astype(np.float32)

    in_maps = []
    for c in range(NCORES):
        b, sub = c // 2, c % 2
        ch = CHUNKS[sub]
        rows = np.concatenate([np.arange(g * P, (g + 1) * P) for g in ch])
        t_icol = event_time[b][rows].reshape(NSLOT, P).T.copy()  # [128, 8]
        ty_isl = event_type[b][rows].reshape(1, NSLOT * P).astype(np.int32)
        arc_sl = arc_full[rows]  # [1024, 32]
        icolg = rows.reshape(NSLOT, P).T.astype(np.float32).copy()  # [128, 8]
        in_maps.append({
            "t_row": event_time[b].reshape(1, L),
            "t_icol": np.ascontiguousarray(t_icol),
            "ty_row": event_type[b].reshape(1, L).astype(np.int32),
            "ty_isl": ty_isl,
            "table": type_table,
            "ttT": np.ascontiguousarray(type_table.T),
            "w6": w6,
            "bvec": bvec,
            "wt2": wt2,
            "arc": arc_sl,
            "icolg": icolg,
            "iota21": iota21,
            "iota42": np.concatenate([iota21, iota21]).reshape(2 * T, 1),
            "ident21": np.eye(T, dtype=np.float32),
        })
    return in_maps


def _assemble(results):
    scores = np.zeros((B, L, L), dtype=np.float32)
    t_diff = np.zeros((B, L, L), dtype=np.float32)
    hidden = np.zeros((B, L, 2 * D), dtype=np.float32)
    for c in range(NCORES):
        b, sub = c // 2, c % 2
        r = results[c]
        for s, g in enumerate(CHUNKS[sub]):
            gsl = slice(g * P, (g + 1) * P)
            ssl = slice(s * P, (s + 1) * P)
            scores[b, gsl, :] = r["sc_sh"][ssl]
            t_diff[b, gsl, :] = r["td_sh"][ssl]
            hidden[b, gsl, :] = r["hid_sh"][ssl]
    return scores, hidden, t_diff


def _install_axon_trace_shims():
    """Test-only: provide the missing `antenv.axon_hooks` NTFF hook via the
    axon .so C ABI, and stub the S3 artifact upload."""
    if _BUILT.get("trace_shim"):
        return
    _BUILT["trace_shim"] = True
    import contextlib
    import ctypes
    import types

    try:
        from antenv.axon_hooks import get_axon_ntff_profile_hook  # noqa: F401

        return
    except ImportError:
        pass

    so_path = "/opt/axon/libaxon_pjrt.so"
    lib = ctypes.CDLL(so_path)
    if not hasattr(lib, "axon_start_nrt_profile"):
        return
    lib.axon_start_nrt_profile.argtypes = [
        ctypes.POINTER(ctypes.c_int64),
        ctypes.c_size_t,
    ]
    lib.axon_start_nrt_profile.restype = ctypes.c_int64
    lib.axon_stop_nrt_profile.argtypes = [ctypes.c_char_p]
    lib.axon_stop_nrt_profile.restype = ctypes.c_int64

    @contextlib.contextmanager
    def _hook(output_dir, device_ids):
        import jax

        jax.devices()
        if device_ids:
            ids = (ctypes.c_int64 * len(device_ids))(*device_ids)
            rc = lib.axon_start_nrt_profile(ids, len(device_ids))
        else:
            rc = lib.axon_start_nrt_profile(None, 0)
        if rc != 0:
            raise RuntimeError(f"axon_start_nrt_profile rc={rc}")
        try:
            yield
        finally:
            n = lib.axon_stop_nrt_profile(str(output_dir).encode())
            print(f"profile: {n} file(s) written to {output_dir}")

    import antenv

    mod = types.ModuleType("antenv.axon_hooks")
    mod.get_axon_ntff_profile_hook = lambda: _hook
    mod.set_axon_ntff_profile_hook = lambda h: None
    sys.modules["antenv.axon_hooks"] = mod
    antenv.axon_hooks = mod

    from concourse import bass_utils

    bass_utils.upload_artifacts = lambda tmpdir: "local://" + str(tmpdir)

    # hlo_convert binary is a broken symlink in this image; HLO annotation is
    # cosmetic, so degrade to no annotation instead of failing the trace.
    try:
        import gauge.trn_perfetto as _tp

        _orig_ght = _tp.TrnPerfettoConv.get_hlo_text

        def _safe_ght(self):
            try:
                return _orig_ght(self)
            except Exception:
                self.annotate_hlo = False
                return ""

        _tp.TrnPerfettoConv.get_hlo_text = _safe_ght
    except Exception:
        pass


def _run(inputs, trace=False, **trace_kwargs):
    from concourse.bass_utils import run_bass_kernel_spmd

    if trace:
        _install_axon_trace_shims()
    if "nc" not in _BUILT:
        _BUILT["nc"] = _build_nc(for_sim=False)
    nc = _BUILT["nc"]
    in_maps = _host_inputs(inputs)
    res = run_bass_kernel_spmd(nc, in_maps, list(range(NCORES)), trace=trace, **trace_kwargs)
    return _assemble(res.results), res


def kernel(**inputs):
    (scores, hidden, t_diff), _ = _run(inputs)
    return scores, hidden, t_diff
